# revision 1
# baseline (speedup 1.0000x reference)
import os

import numpy as np

from concourse import bass, bass_utils, mybir

# Problem constants (hardcoded per contract: kernel.py is self-contained)
N_USERS = 50000
K = 2016          # skew-vector length for D=64
D = 64
B = 8192
NCORES = 8
R = N_USERS // NCORES   # 6250 rows owned per core
CAP = 1280              # routed-pair capacity per core (expected ~1024)
P = 128
NT = CAP // P           # index tiles per core
CHUNK = 125             # bulk-copy chunk rows; 6250 = 50 * 125
NCHUNK = R // CHUNK
ETA = 0.05
RADIUS = 0.693

_IU = np.triu_indices(D, 1)

LAST_EXEC_NS = None
_NC_CACHE = {}


def _spec_norm(A):
    # A: (B, D, D) skew -> largest singular value via eigvalsh(-A@A)
    M = -np.matmul(A, A)
    ev = np.linalg.eigvalsh(M)
    return np.sqrt(np.maximum(ev[:, -1], 0.0))


def _host_w(fib, uid, delta):
    """Per-routed-row additive update w s.t. new_row = old_row + w (exact
    reference math, float64 interior)."""
    rows_old = fib[uid].astype(np.float64)
    A = np.zeros((uid.shape[0], D, D), np.float64)
    A[:, _IU[0], _IU[1]] = rows_old
    A = A - A.transpose(0, 2, 1)
    dA = 0.5 * (delta.astype(np.float64) - delta.astype(np.float64).transpose(0, 2, 1))
    # scale == 1 whenever RADIUS - sigma_old >= eta*sigma_del; sigma <= fro
    # makes the Frobenius test a sufficient condition. Exact eigvalsh only
    # for rows the cheap bound can't settle.
    fro_A = np.sqrt((A * A).sum(axis=(1, 2)))
    fro_dA = ETA * np.sqrt((dA * dA).sum(axis=(1, 2)))
    scale = np.ones(uid.shape[0])
    hard = (RADIUS - fro_A) < (fro_dA + 1e-6)
    if hard.any():
        s_old = _spec_norm(A[hard])
        s_del = ETA * _spec_norm(dA[hard])
        avail = np.clip(RADIUS - s_old, 1e-8, None)
        scale[hard] = np.minimum(avail / (s_del + 1e-8), 1.0)
    dAs = dA * scale[:, None, None]
    A_new = A + ETA * dAs + 0.5 * ETA * (np.matmul(A, dAs) - np.matmul(dAs, A))
    A_new = 0.5 * (A_new - A_new.transpose(0, 2, 1))
    fro_new = np.sqrt((A_new * A_new).sum(axis=(1, 2)))
    hard2 = fro_new > (RADIUS - 1e-6)
    if hard2.any():
        s_new = _spec_norm(A_new[hard2])
        A_new[hard2] *= np.minimum(RADIUS / (s_new + 1e-8), 1.0)[:, None, None]
    new_rows = A_new[:, _IU[0], _IU[1]].astype(np.float32)
    return new_rows - fib[uid]


NFULL = R // P          # 48 full 128-row copy chunks
TAIL = R - NFULL * P    # 106 tail rows


def _build_nc():
    nc = bass.Bass()
    fib = nc.dram_tensor("fib", [R, K], mybir.dt.float32, kind="ExternalInput")
    idx = nc.dram_tensor("idx", [P, NT], mybir.dt.int32, kind="ExternalInput")
    wvec = nc.dram_tensor("wvec", [CAP, K], mybir.dt.float32, kind="ExternalInput")
    out = nc.dram_tensor("out", [R, K], mybir.dt.float32, kind="ExternalOutput")

    NBUF = 4
    NCH = NFULL + 1  # 48 full chunks + tail

    with (
        nc.sbuf_tensor([P, NBUF * K], mybir.dt.float32) as cbuf,
        nc.sbuf_tensor([P, NT * K], mybir.dt.float32) as w_sb,
        nc.sbuf_tensor([P, NT], mybir.dt.int32) as i_sb,
        nc.semaphore() as s_stage,
        nc.semaphore() as s_load,
        nc.semaphore() as s_store,
        nc.semaphore() as s_scat,
        nc.Block() as block,
    ):
        def chunk(ci):
            lo = ci * P
            hi = min(lo + P, R)
            return lo, hi, hi - lo

        @block.sync
        def _(sync):
            # Stage update vectors + indices into SBUF.
            sync.dma_start(
                out=w_sb[:, :].rearrange("p (t k) -> p t k", k=K),
                in_=wvec[:, :].rearrange("(t p) k -> p t k", p=P),
            ).then_inc(s_stage, 16)
            sync.dma_start(out=i_sb[:, :], in_=idx[:, :]).then_inc(s_stage, 16)
            # Bulk-copy loads (stores run on scalar's separate HWDGE FIFO).
            for ci in range(NCH):
                lo, hi, n = chunk(ci)
                if ci >= NBUF:
                    # WAR: slot reused, wait until its store drained.
                    sync.wait_ge(s_store, 16 * (ci - NBUF + 1))
                b = ci % NBUF
                sync.dma_start(
                    out=cbuf[:n, b * K:(b + 1) * K], in_=fib[lo:hi, :]
                ).then_inc(s_load, 16)

        @block.scalar
        def _(scalar):
            for ci in range(NCH):
                lo, hi, n = chunk(ci)
                b = ci % NBUF
                scalar.wait_ge(s_load, 16 * (ci + 1))
                scalar.dma_start(
                    out=out[lo:hi, :], in_=cbuf[:n, b * K:(b + 1) * K]
                ).then_inc(s_store, 16)

        @block.gpsimd
        def _(gp):
            gp.wait_ge(s_stage, 32)
            gp.wait_ge(s_store, 16 * NCH)  # all copy writes landed
            # Scatter-accumulate w onto owned rows (new = old + w).
            # Padded indices (== R) are bounds-skipped.
            for t in range(NT):
                gp.indirect_dma_start(
                    out=out[:],
                    out_offset=bass.IndirectOffsetOnAxis(
                        ap=i_sb[:, t:t + 1], axis=0
                    ),
                    in_=w_sb[:, t * K:(t + 1) * K],
                    in_offset=None,
                    bounds_check=R - 1,
                    oob_is_err=False,
                    compute_op=mybir.AluOpType.add,
                ).then_inc(s_scat, 16)
            gp.wait_ge(s_scat, 16 * NT)
    return nc


def kernel(**inputs):
    global LAST_EXEC_NS
    fib = np.ascontiguousarray(inputs["fiber_vectors"], dtype=np.float32)
    uid = np.asarray(inputs["user_ids"], dtype=np.int32)
    delta = np.ascontiguousarray(inputs["delta_A"], dtype=np.float32)

    w = _host_w(fib, uid, delta)

    owner = uid // R
    local = (uid - owner * R).astype(np.int32)
    in_maps = []
    for c in range(NCORES):
        m = owner == c
        cnt = int(m.sum())
        assert cnt <= CAP, f"shard {c} overflow: {cnt} > {CAP}"
        idx_pad = np.full((CAP,), R, np.int32)  # R == OOB sentinel, skipped
        w_pad = np.zeros((CAP, K), np.float32)
        idx_pad[:cnt] = local[m]
        w_pad[:cnt] = w[m]
        # device expects idx as [P, NT] with [p, t] = entry t*P+p
        idx_dev = np.ascontiguousarray(idx_pad.reshape(NT, P).T)
        in_maps.append(
            {"fib": fib[c * R:(c + 1) * R], "idx": idx_dev, "wvec": w_pad}
        )

    if "nc" not in _NC_CACHE:
        _NC_CACHE["nc"] = _build_nc()
    nc = _NC_CACHE["nc"]

    res = bass_utils.run_bass_kernel_spmd(
        nc,
        in_maps,
        core_ids=list(range(NCORES)),
        trace=os.environ.get("KERNEL_TRACE", "0") == "1",
    )
    LAST_EXEC_NS = res.exec_time_ns
    return np.concatenate([res.results[c]["out"] for c in range(NCORES)], axis=0)



# revision 6
# speedup vs baseline: 1.9513x; 1.9513x over previous
import dataclasses
import os

import numpy as np
import ml_dtypes

from concourse import bass, bass_utils, mybir

bf16 = ml_dtypes.bfloat16

# Problem constants (hardcoded: kernel.py must be self-contained)
D = 64
K = D * (D - 1) // 2     # 2016 triu vec length
S = 2048                 # slot-packed length: 32 slots x 64
M = D * D                # 4096 flat matrix
B = 8192
NCORES = 8
RB = B // NCORES         # 1024 rows per core
P = 128                  # partitions
NT = RB // P             # 8 row-tiles per core
G = 2                    # tiles per compute group
NG = NT // G             # 4 groups
ETA = 0.05
RADIUS = 0.693

_IU = np.triu_indices(D, 1)

LAST_EXEC_NS = None
_NC_CACHE = {}

# ---------------------------------------------------------------------------
# Packing tables. Slot s (s=0..31, width 64) holds strip s (row s, cols
# s+1..63: 63-s values) followed by strip 62-s (s+1 values); slot 31 is
# strip 31 + 32 pad. Total 2048 (vs 2016 vec) but every slot is fixed-width,
# which makes both unvec directions two rectangular strided copies.
# ---------------------------------------------------------------------------
_off = np.zeros(D, np.int64)
for _i in range(1, D):
    _off[_i] = _off[_i - 1] + (D - _i)

IDX_PACK = np.zeros(S, np.int64)
_valid = np.zeros(S, bool)
for _s in range(32):
    _L1 = 63 - _s
    IDX_PACK[_s * 64:_s * 64 + _L1] = _off[_s] + np.arange(_L1)
    _valid[_s * 64:_s * 64 + _L1] = True
    if _s < 31:
        _t = 62 - _s
        IDX_PACK[_s * 64 + _L1:_s * 64 + 64] = _off[_t] + np.arange(_s + 1)
        _valid[_s * 64 + _L1:_s * 64 + 64] = True

INV = np.zeros(K, np.int64)
INV[IDX_PACK[_valid]] = np.nonzero(_valid)[0]

# upper-triangular mask over flat [64,64], replicated across 128 partitions
_mup = np.zeros(M, np.float32)
_mup[_IU[0] * D + _IU[1]] = 1.0
MASK_UP = np.ascontiguousarray(np.broadcast_to(np.tile(_mup, G), (P, G * M))).astype(bf16)

# select mask: position (s,e) valid for part1 iff e < 63-s
_msel = np.zeros(S, np.float32)
for _s in range(32):
    _msel[_s * 64:_s * 64 + (63 - _s)] = 1.0
MASK_SEL = np.ascontiguousarray(np.broadcast_to(np.tile(_msel, G), (P, G * S))).astype(bf16)


def _ap(base, ap_dims, offset):
    """Custom strided AP over an SBUF tensor's flat [P, n] view."""
    return dataclasses.replace(base, ap=[base.ap[0]] + ap_dims, offset=offset)


def _build_nc():
    nc = bass.Bass()
    vp = nc.dram_tensor("vp", [RB, S], mybir.dt.bfloat16, kind="ExternalInput")
    dp = nc.dram_tensor("dp", [RB, S], mybir.dt.bfloat16, kind="ExternalInput")
    mup = nc.dram_tensor("mup", [P, G * M], mybir.dt.bfloat16, kind="ExternalInput")
    msel = nc.dram_tensor("msel", [P, G * S], mybir.dt.bfloat16, kind="ExternalInput")
    wp = nc.dram_tensor("wp", [RB, S], mybir.dt.bfloat16, kind="ExternalOutput")

    dt = mybir.dt.bfloat16
    mult = mybir.AluOpType.mult
    add = mybir.AluOpType.add
    sub = mybir.AluOpType.subtract

    with (
        nc.sbuf_tensor([P, G * M], dt) as sMu,
        nc.sbuf_tensor([P, G * S], dt) as sMs,
        nc.sbuf_tensor([P, G * S], dt) as Vb,
        nc.sbuf_tensor([P, G * S], dt) as Db,
        nc.sbuf_tensor([P, G * S], dt) as Wv,
        nc.sbuf_tensor([P, G * M], dt) as UA,   # Up_A, later MAC tmp
        nc.sbuf_tensor([P, G * M], dt) as UD,   # Up_D, later P accumulator
        nc.sbuf_tensor([P, G * M], dt) as TA,   # A, later W
        nc.sbuf_tensor([P, G * M], dt) as TD,   # D
        nc.semaphore() as s_in,
        nc.semaphore() as s_mask,
        nc.semaphore() as s_v,
        nc.semaphore() as s_out,
        nc.Block() as block,
    ):
        # flat + structured views
        def tview(t, inner):  # [P, G*inner] -> [P, G, inner]
            return t[:, :].rearrange("p (g e) -> p g e", e=inner)

        Vb4 = Vb[:, :].rearrange("p (g s e) -> p g s e", s=32, e=64)
        Db4 = Db[:, :].rearrange("p (g s e) -> p g s e", s=32, e=64)
        Wv4 = Wv[:, :].rearrange("p (g s e) -> p g s e", s=32, e=64)

        A4 = TA[:, :].rearrange("p (g i j) -> p g i j", i=D, j=D)
        D4 = TD[:, :].rearrange("p (g i j) -> p g i j", i=D, j=D)
        P4 = UD[:, :].rearrange("p (g i j) -> p g i j", i=D, j=D)
        T4 = UA[:, :].rearrange("p (g i j) -> p g i j", i=D, j=D)

        # unvec target views on a flat base
        UAf = UA[:, :]
        UDf = UD[:, :]
        TAf = TA[:, :]

        @block.sync
        def _(sync):
            sync.dma_start(out=sMu[:, :], in_=mup[:, :]).then_inc(s_mask, 16)
            sync.dma_start(out=sMs[:, :], in_=msel[:, :]).then_inc(s_mask, 16)
            for g in range(NG):
                if g > 0:
                    sync.wait_ge(s_v, g)  # vector done with Vb/Db of group g-1
                rows = slice(g * G * P, (g + 1) * G * P)
                sync.dma_start(
                    out=tview(Vb, S),
                    in_=vp[rows, :].rearrange("(g p) e -> p g e", p=P),
                ).then_inc(s_in, 16)
                sync.dma_start(
                    out=tview(Db, S),
                    in_=dp[rows, :].rearrange("(g p) e -> p g e", p=P),
                ).then_inc(s_in, 16)

        @block.vector
        def _(vector):
            def dr():
                vector.drain()

            def unvec(upf, up_struct, src4):
                # memset; part2 (rows 31..62 full-width; strip-s dup into
                # lower, masked later); then part1 (rows 0..31 upper; also
                # fixes row 31 upper over part2's slot-31 pad garbage).
                vector.memset(upf, 0.0)
                dr()
                out_p2 = up_struct[:, :, 31:63, :]          # [p g 32 64] rows 31..62
                in_p2 = src4[:, :, 31::-1, :]               # slots 31..0
                vector.tensor_copy(out_p2, in_p2)
                dr()
                out_p1 = _ap(upf, [[M, G], [65, 32], [1, 64]], 1)
                in_p1 = src4
                vector.tensor_copy(out_p1, in_p1)
                dr()
                # mask to strictly-upper
                vector.tensor_tensor(upf, upf, sMu[:, :], mult)
                dr()

            UPA = UA[:, :].rearrange("p (g r e) -> p g r e", r=D, e=D)
            UPD = UD[:, :].rearrange("p (g r e) -> p g r e", r=D, e=D)

            vector.wait_ge(s_mask, 32)
            for g in range(NG):
                vector.wait_ge(s_in, 32 * (g + 1))
                if g > 0:
                    vector.wait_ge(s_out, 16 * g)  # prior store drained

                unvec(UA[:, :], UPA, Vb4)
                # A = Up - Up^T
                ua = UA[:, :].rearrange("p (g i j) -> p g i j", i=D, j=D)
                vector.tensor_tensor(A4, ua, ua.transpose([0, 1, 3, 2]), sub)
                dr()
                unvec(UD[:, :], UPD, Db4)
                ud = UD[:, :].rearrange("p (g i j) -> p g i j", i=D, j=D)
                vector.tensor_tensor(D4, ud, ud.transpose([0, 1, 3, 2]), sub)
                dr()

                # MAC: P = A @ D, accumulated over k. UD is dead -> P, UA -> tmp.
                a0 = A4[:, :, :, 0].unsqueeze(3).broadcast_to([P, G, D, D])
                d0 = D4[:, :, 0, :].unsqueeze(2).broadcast_to([P, G, D, D])
                vector.tensor_tensor(P4, a0, d0, mult)
                dr()
                with vector.Fori(1, D) as k:
                    ak = A4[:, :, :, k].unsqueeze(3).broadcast_to([P, G, D, D])
                    dk = D4[:, :, k, :].unsqueeze(2).broadcast_to([P, G, D, D])
                    vector.tensor_tensor(T4, ak, dk, mult)
                    dr()
                    vector.tensor_tensor(P4, P4, T4, add)
                    dr()

                # W = P - P^T -> TA (A dead); prescale by eta/2
                vector.tensor_tensor(A4, P4, P4.transpose([0, 1, 3, 2]), sub)
                dr()
                vector.tensor_scalar_mul(TA[:, :], TA[:, :], 0.5 * ETA)
                dr()
                # extract: t2 -> Wv, t1 -> Vb (dead), predicated merge
                in_t2 = _ap(TAf, [[M, G], [-64, 32], [1, 64]], 3968)
                vector.tensor_copy(Wv4, in_t2)
                in_t1 = _ap(TAf, [[M, G], [65, 32], [1, 64]], 1)
                vector.tensor_copy(Vb4, in_t1)
                dr()
                vector.copy_predicated(
                    Wv[:, :], sMs[:, :].bitcast(mybir.dt.uint16), Vb[:, :]
                )
                dr()
                # w = eta * dvec + Wv
                wv = tview(Wv, S)
                db = tview(Db, S)
                vector.scalar_tensor_tensor(
                    wv, db, ETA, wv, mult, add
                ).then_inc(s_v, 1)

        @block.scalar
        def _(scalar):
            for g in range(NG):
                scalar.wait_ge(s_v, g + 1)
                rows = slice(g * G * P, (g + 1) * G * P)
                scalar.dma_start(
                    out=wp[rows, :].rearrange("(g p) e -> p g e", p=P),
                    in_=tview(Wv, S),
                ).then_inc(s_out, 16)

    return nc


def _exact_w_rows(vrows, drows):
    """Exact reference math (float64) for rows the cheap bounds can't settle.
    vrows: [n, K] f32 triu-vec of A_old; drows: [n, K] f32 triu-vec of dA.
    Returns w rows [n, K] f32 with w = vec(A_new) - vrows."""
    n = vrows.shape[0]
    A = np.zeros((n, D, D), np.float64)
    A[:, _IU[0], _IU[1]] = vrows.astype(np.float64)
    A -= A.transpose(0, 2, 1)
    dA = np.zeros((n, D, D), np.float64)
    dA[:, _IU[0], _IU[1]] = drows.astype(np.float64)
    dA -= dA.transpose(0, 2, 1)
    s_old = np.linalg.svd(A, compute_uv=False)[:, 0:1, None]
    s_del = np.linalg.svd(ETA * dA, compute_uv=False)[:, 0:1, None]
    avail = np.clip(RADIUS - s_old, 1e-8, None)
    scale = np.minimum(avail / (s_del + 1e-8), 1.0)
    dAs = dA * scale
    A_new = A + ETA * dAs + 0.5 * ETA * (np.matmul(A, dAs) - np.matmul(dAs, A))
    A_new = 0.5 * (A_new - A_new.transpose(0, 2, 1))
    s_new = np.linalg.svd(A_new, compute_uv=False)[:, 0:1, None]
    A_new = A_new * np.minimum(RADIUS / (s_new + 1e-8), 1.0)
    return (A_new[:, _IU[0], _IU[1]] - vrows.astype(np.float64)).astype(np.float32)


def kernel(**inputs):
    global LAST_EXEC_NS
    fib = np.ascontiguousarray(inputs["fiber_vectors"], dtype=np.float32)
    uid = np.asarray(inputs["user_ids"], dtype=np.int64)
    delta = np.ascontiguousarray(inputs["delta_A"], dtype=np.float32)

    # host: gather + skew-project
    V = fib[uid]                                                    # [B, K]
    dvec = 0.5 * (delta[:, _IU[0], _IU[1]] - delta[:, _IU[1], _IU[0]])

    # rows where the Frobenius sufficient conditions can't prove that both
    # the BCH scale and the final clamp are exactly 1 -> exact host path
    fro_A = np.sqrt(2.0) * np.linalg.norm(V, axis=1)
    fro_dAe = ETA * np.sqrt(2.0) * np.linalg.norm(dvec, axis=1)
    hard = ((RADIUS - fro_A) < fro_dAe + 1e-6) | (
        fro_A + fro_dAe + fro_A * fro_dAe > RADIUS - 1e-6
    )

    # slot-pack to bf16
    vp = V[:, IDX_PACK].astype(bf16)
    dp = dvec[:, IDX_PACK].astype(bf16)

    in_maps = []
    for c in range(NCORES):
        rows = slice(c * RB, (c + 1) * RB)
        in_maps.append({
            "vp": vp[rows], "dp": dp[rows], "mup": MASK_UP, "msel": MASK_SEL,
        })

    if "nc" not in _NC_CACHE:
        _NC_CACHE["nc"] = _build_nc()
    nc = _NC_CACHE["nc"]

    res = bass_utils.run_bass_kernel_spmd(
        nc,
        in_maps,
        core_ids=list(range(NCORES)),
        trace=os.environ.get("KERNEL_TRACE", "0") == "1",
    )
    LAST_EXEC_NS = res.exec_time_ns

    wpk = np.concatenate(
        [np.asarray(res.results[c]["wp"]) for c in range(NCORES)], axis=0
    ).astype(np.float32)
    wvec = wpk[:, INV]                                              # unpack

    if hard.any():
        hidx = np.nonzero(hard)[0]
        wvec[hidx] = _exact_w_rows(V[hidx], dvec[hidx])

    out = fib.copy()
    out[uid] = V + wvec
    return out


# revision 8
# speedup vs baseline: 2.7045x; 1.3860x over previous
import dataclasses
import os

import numpy as np
import ml_dtypes

from concourse import bass, bass_utils, mybir

bf16 = ml_dtypes.bfloat16

# Problem constants (hardcoded: kernel.py must be self-contained)
D = 64
K = D * (D - 1) // 2     # 2016 triu vec length
S = 2048                 # slot-packed length: 32 slots x 64
M = D * D                # 4096 flat matrix
B = 8192
NCORES = 8
RB = B // NCORES         # 1024 rows per core
P = 128                  # partitions
NT = RB // P             # 8 row-tiles per core
G = 2                    # tiles per compute group
NG = NT // G             # 4 groups
ETA = 0.05
RADIUS = 0.693

_IU = np.triu_indices(D, 1)

LAST_EXEC_NS = None
_NC_CACHE = {}

# ---------------------------------------------------------------------------
# Packing tables. Slot s (s=0..31, width 64) holds strip s (row s, cols
# s+1..63: 63-s values) followed by strip 62-s (s+1 values); slot 31 is
# strip 31 + 32 pad. Total 2048 (vs 2016 vec) but every slot is fixed-width,
# which makes both unvec directions two rectangular strided copies.
# ---------------------------------------------------------------------------
_off = np.zeros(D, np.int64)
for _i in range(1, D):
    _off[_i] = _off[_i - 1] + (D - _i)

IDX_PACK = np.zeros(S, np.int64)
_valid = np.zeros(S, bool)
for _s in range(32):
    _L1 = 63 - _s
    IDX_PACK[_s * 64:_s * 64 + _L1] = _off[_s] + np.arange(_L1)
    _valid[_s * 64:_s * 64 + _L1] = True
    if _s < 31:
        _t = 62 - _s
        IDX_PACK[_s * 64 + _L1:_s * 64 + 64] = _off[_t] + np.arange(_s + 1)
        _valid[_s * 64 + _L1:_s * 64 + 64] = True

INV = np.zeros(K, np.int64)
INV[IDX_PACK[_valid]] = np.nonzero(_valid)[0]

# upper-triangular mask over flat [64,64], replicated across 128 partitions
_mup = np.zeros(M, np.float32)
_mup[_IU[0] * D + _IU[1]] = 1.0
MASK_UP = np.ascontiguousarray(np.broadcast_to(np.tile(_mup, G), (P, G * M))).astype(bf16)

# select mask: position (s,e) valid for part1 iff e < 63-s
_msel = np.zeros(S, np.float32)
for _s in range(32):
    _msel[_s * 64:_s * 64 + (63 - _s)] = 1.0
MASK_SEL = np.ascontiguousarray(np.broadcast_to(np.tile(_msel, G), (P, G * S))).astype(bf16)


def _ap(base, ap_dims, offset):
    """Custom strided AP over an SBUF tensor's flat [P, n] view."""
    return dataclasses.replace(base, ap=[base.ap[0]] + ap_dims, offset=offset)


def _build_nc():
    nc = bass.Bass()
    vp = nc.dram_tensor("vp", [RB, S], mybir.dt.bfloat16, kind="ExternalInput")
    dp = nc.dram_tensor("dp", [RB, S], mybir.dt.bfloat16, kind="ExternalInput")
    mup = nc.dram_tensor("mup", [P, G * M], mybir.dt.bfloat16, kind="ExternalInput")
    msel = nc.dram_tensor("msel", [P, G * S], mybir.dt.bfloat16, kind="ExternalInput")
    wp = nc.dram_tensor("wp", [RB, S], mybir.dt.bfloat16, kind="ExternalOutput")

    dt = mybir.dt.bfloat16
    mult = mybir.AluOpType.mult
    add = mybir.AluOpType.add
    sub = mybir.AluOpType.subtract

    with (
        nc.sbuf_tensor([P, G * M], dt) as sMu,
        nc.sbuf_tensor([P, G * S], dt) as sMs,
        nc.sbuf_tensor([P, G * S], dt) as Vb,
        nc.sbuf_tensor([P, G * S], dt) as Db,
        nc.sbuf_tensor([P, G * S], dt) as Wv,
        nc.sbuf_tensor([P, G * M], dt) as UA,   # Up_A, later MAC tmp
        nc.sbuf_tensor([P, G * M], dt) as UD,   # Up_D, later P accumulator
        nc.sbuf_tensor([P, G * M], dt) as TA,   # A, later W
        nc.sbuf_tensor([P, G * M], dt) as TD,   # D
        nc.semaphore() as s_in,
        nc.semaphore() as s_mask,
        nc.semaphore() as s_v,
        nc.semaphore() as s_out,
        nc.Block() as block,
    ):
        # flat + structured views
        def tview(t, inner):  # [P, G*inner] -> [P, G, inner]
            return t[:, :].rearrange("p (g e) -> p g e", e=inner)

        Vb4 = Vb[:, :].rearrange("p (g s e) -> p g s e", s=32, e=64)
        Db4 = Db[:, :].rearrange("p (g s e) -> p g s e", s=32, e=64)
        Wv4 = Wv[:, :].rearrange("p (g s e) -> p g s e", s=32, e=64)

        A4 = TA[:, :].rearrange("p (g i j) -> p g i j", i=D, j=D)
        D4 = TD[:, :].rearrange("p (g i j) -> p g i j", i=D, j=D)
        P4 = UD[:, :].rearrange("p (g i j) -> p g i j", i=D, j=D)
        T4 = UA[:, :].rearrange("p (g i j) -> p g i j", i=D, j=D)

        # unvec target views on a flat base
        UAf = UA[:, :]
        UDf = UD[:, :]
        TAf = TA[:, :]

        @block.sync
        def _(sync):
            sync.dma_start(out=sMu[:, :], in_=mup[:, :]).then_inc(s_mask, 16)
            sync.dma_start(out=sMs[:, :], in_=msel[:, :]).then_inc(s_mask, 16)
            for g in range(NG):
                if g > 0:
                    sync.wait_ge(s_v, g)  # vector done with Vb/Db of group g-1
                rows = slice(g * G * P, (g + 1) * G * P)
                sync.dma_start(
                    out=tview(Vb, S),
                    in_=vp[rows, :].rearrange("(g p) e -> p g e", p=P),
                ).then_inc(s_in, 16)
                sync.dma_start(
                    out=tview(Db, S),
                    in_=dp[rows, :].rearrange("(g p) e -> p g e", p=P),
                ).then_inc(s_in, 16)

        @block.vector
        def _(vector):
            def dr():
                vector.drain()

            def unvec(upf, up_struct, src4):
                # memset; part2 (rows 31..62 full-width; strip-s dup into
                # lower, masked later); then part1 (rows 0..31 upper; also
                # fixes row 31 upper over part2's slot-31 pad garbage).
                vector.memset(upf, 0.0)
                dr()
                out_p2 = up_struct[:, :, 31:63, :]          # [p g 32 64] rows 31..62
                in_p2 = src4[:, :, 31::-1, :]               # slots 31..0
                vector.tensor_copy(out_p2, in_p2)
                dr()
                out_p1 = _ap(upf, [[M, G], [65, 32], [1, 64]], 1)
                in_p1 = src4
                vector.tensor_copy(out_p1, in_p1)
                dr()
                # mask to strictly-upper
                vector.tensor_tensor(upf, upf, sMu[:, :], mult)
                dr()

            UPA = UA[:, :].rearrange("p (g r e) -> p g r e", r=D, e=D)
            UPD = UD[:, :].rearrange("p (g r e) -> p g r e", r=D, e=D)

            vector.wait_ge(s_mask, 32)
            for g in range(NG):
                vector.wait_ge(s_in, 32 * (g + 1))
                if g > 0:
                    vector.wait_ge(s_out, 16 * g)  # prior store drained

                unvec(UA[:, :], UPA, Vb4)
                # A = Up - Up^T
                ua = UA[:, :].rearrange("p (g i j) -> p g i j", i=D, j=D)
                vector.tensor_tensor(A4, ua, ua.transpose([0, 1, 3, 2]), sub)
                dr()
                unvec(UD[:, :], UPD, Db4)
                ud = UD[:, :].rearrange("p (g i j) -> p g i j", i=D, j=D)
                vector.tensor_tensor(D4, ud, ud.transpose([0, 1, 3, 2]), sub)
                dr()

                # MAC: P = A @ D, accumulated over k. UD is dead -> P, UA -> tmp.
                a0 = A4[:, :, :, 0].unsqueeze(3).broadcast_to([P, G, D, D])
                d0 = D4[:, :, 0, :].unsqueeze(2).broadcast_to([P, G, D, D])
                vector.tensor_tensor(P4, a0, d0, mult)
                dr()
                with vector.Fori(1, D) as k:
                    ak = A4[:, :, :, k].unsqueeze(3).broadcast_to([P, G, D, D])
                    dk = D4[:, :, k, :].unsqueeze(2).broadcast_to([P, G, D, D])
                    vector.tensor_tensor(T4, ak, dk, mult)
                    dr()
                    vector.tensor_tensor(P4, P4, T4, add)
                    dr()

                # W = P - P^T -> TA (A dead); prescale by eta/2
                vector.tensor_tensor(A4, P4, P4.transpose([0, 1, 3, 2]), sub)
                dr()
                vector.tensor_scalar_mul(TA[:, :], TA[:, :], 0.5 * ETA)
                dr()
                # extract: t2 -> Wv, t1 -> Vb (dead), predicated merge
                in_t2 = _ap(TAf, [[M, G], [-64, 32], [1, 64]], 3968)
                vector.tensor_copy(Wv4, in_t2)
                in_t1 = _ap(TAf, [[M, G], [65, 32], [1, 64]], 1)
                vector.tensor_copy(Vb4, in_t1)
                dr()
                vector.copy_predicated(
                    Wv[:, :], sMs[:, :].bitcast(mybir.dt.uint16), Vb[:, :]
                )
                dr()
                # w = eta * dvec + Wv
                wv = tview(Wv, S)
                db = tview(Db, S)
                vector.scalar_tensor_tensor(
                    wv, db, ETA, wv, mult, add
                ).then_inc(s_v, 1)

        @block.scalar
        def _(scalar):
            for g in range(NG):
                scalar.wait_ge(s_v, g + 1)
                rows = slice(g * G * P, (g + 1) * G * P)
                scalar.dma_start(
                    out=wp[rows, :].rearrange("(g p) e -> p g e", p=P),
                    in_=tview(Wv, S),
                ).then_inc(s_out, 16)

    return nc


def _get_runner():
    """Build (once) a cached jitted SPMD executor with device-resident masks
    and output-zero buffer. Re-jitting per call (as run_bass_kernel_spmd
    does) costs seconds of XLA compile + buffer churn per invocation."""
    if "fn" in _NC_CACHE:
        return _NC_CACHE["fn"]
    import jax
    import jax.numpy as jnp
    from jax.sharding import Mesh, NamedSharding, PartitionSpec
    from jax.experimental.shard_map import shard_map
    from concourse import bass2jax

    bass2jax.install_neuronx_cc_hook()
    if "nc" not in _NC_CACHE:
        _NC_CACHE["nc"] = _build_nc()
    nc = _NC_CACHE["nc"]

    out_avals = (jax.core.ShapedArray((RB, S), jnp.bfloat16),)
    in_names = ("vp", "dp", "mup", "msel", "wp", "partition_id")

    def _body(vp_, dp_, mu_, ms_, wz_):
        outs = bass2jax._bass_exec_p.bind(
            vp_, dp_, mu_, ms_, wz_, bass2jax.partition_id_tensor(),
            out_avals=out_avals,
            in_names=in_names,
            out_names=("wp",),
            lowering_input_output_aliases=(),
            sim_require_finite=True,
            sim_require_nnan=True,
            nc=nc,
        )
        return outs[0]

    devices = jax.devices()[:NCORES]
    mesh = Mesh(np.asarray(devices), ("core",))
    spec = PartitionSpec("core")
    sh = NamedSharding(mesh, spec)
    sm = shard_map(
        _body, mesh=mesh, in_specs=(spec,) * 5, out_specs=spec, check_rep=False
    )
    fn = jax.jit(sm, in_shardings=(sh,) * 5, out_shardings=sh)
    mup_d = jax.device_put(np.tile(MASK_UP, (NCORES, 1)), sh)
    msel_d = jax.device_put(np.tile(MASK_SEL, (NCORES, 1)), sh)
    wz_d = jax.device_put(np.zeros((NCORES * RB, S), bf16), sh)
    _NC_CACHE["fn"] = (fn, mup_d, msel_d, wz_d)
    return _NC_CACHE["fn"]


def _exact_w_rows(vrows, drows):
    """Exact reference math (float64) for rows the cheap bounds can't settle.
    vrows: [n, K] f32 triu-vec of A_old; drows: [n, K] f32 triu-vec of dA.
    Returns w rows [n, K] f32 with w = vec(A_new) - vrows."""
    n = vrows.shape[0]
    A = np.zeros((n, D, D), np.float64)
    A[:, _IU[0], _IU[1]] = vrows.astype(np.float64)
    A -= A.transpose(0, 2, 1)
    dA = np.zeros((n, D, D), np.float64)
    dA[:, _IU[0], _IU[1]] = drows.astype(np.float64)
    dA -= dA.transpose(0, 2, 1)
    s_old = np.linalg.svd(A, compute_uv=False)[:, 0:1, None]
    s_del = np.linalg.svd(ETA * dA, compute_uv=False)[:, 0:1, None]
    avail = np.clip(RADIUS - s_old, 1e-8, None)
    scale = np.minimum(avail / (s_del + 1e-8), 1.0)
    dAs = dA * scale
    A_new = A + ETA * dAs + 0.5 * ETA * (np.matmul(A, dAs) - np.matmul(dAs, A))
    A_new = 0.5 * (A_new - A_new.transpose(0, 2, 1))
    s_new = np.linalg.svd(A_new, compute_uv=False)[:, 0:1, None]
    A_new = A_new * np.minimum(RADIUS / (s_new + 1e-8), 1.0)
    return (A_new[:, _IU[0], _IU[1]] - vrows.astype(np.float64)).astype(np.float32)


def kernel(**inputs):
    global LAST_EXEC_NS
    fib = np.ascontiguousarray(inputs["fiber_vectors"], dtype=np.float32)
    uid = np.asarray(inputs["user_ids"], dtype=np.int64)
    delta = np.ascontiguousarray(inputs["delta_A"], dtype=np.float32)

    # host: gather + skew-project
    V = fib[uid]                                                    # [B, K]
    dvec = 0.5 * (delta[:, _IU[0], _IU[1]] - delta[:, _IU[1], _IU[0]])

    # rows where the Frobenius sufficient conditions can't prove that both
    # the BCH scale and the final clamp are exactly 1 -> exact host path
    fro_A = np.sqrt(2.0) * np.linalg.norm(V, axis=1)
    fro_dAe = ETA * np.sqrt(2.0) * np.linalg.norm(dvec, axis=1)
    hard = ((RADIUS - fro_A) < fro_dAe + 1e-6) | (
        fro_A + fro_dAe + fro_A * fro_dAe > RADIUS - 1e-6
    )

    # slot-pack to bf16
    vp = V[:, IDX_PACK].astype(bf16)
    dp = dvec[:, IDX_PACK].astype(bf16)

    if os.environ.get("KERNEL_TRACE", "0") == "1":
        # profiling path: per-call jit, but produces an NTFF trace
        if "nc" not in _NC_CACHE:
            _NC_CACHE["nc"] = _build_nc()
        in_maps = []
        for c in range(NCORES):
            rows = slice(c * RB, (c + 1) * RB)
            in_maps.append({
                "vp": vp[rows], "dp": dp[rows],
                "mup": MASK_UP, "msel": MASK_SEL,
            })
        res = bass_utils.run_bass_kernel_spmd(
            _NC_CACHE["nc"], in_maps, core_ids=list(range(NCORES)), trace=True,
        )
        LAST_EXEC_NS = res.exec_time_ns
        wpk = np.concatenate(
            [np.asarray(res.results[c]["wp"]) for c in range(NCORES)], axis=0
        ).astype(np.float32)
    else:
        fn, mup_d, msel_d, wz_d = _get_runner()
        outg = fn(vp, dp, mup_d, msel_d, wz_d)
        wpk = np.asarray(outg).astype(np.float32)
        LAST_EXEC_NS = None

    wvec = wpk[:, INV]                                              # unpack

    if hard.any():
        hidx = np.nonzero(hard)[0]
        wvec[hidx] = _exact_w_rows(V[hidx], dvec[hidx])

    out = fib.copy()
    out[uid] = V + wvec
    return out


# revision 10
# speedup vs baseline: 6.0838x; 2.2495x over previous
import dataclasses
import os

import numpy as np
import ml_dtypes

from concourse import bass, bass_utils, mybir

bf16 = ml_dtypes.bfloat16

# Problem constants (hardcoded: kernel.py must be self-contained)
D = 64
K = D * (D - 1) // 2     # 2016 triu vec length
S = 2048                 # slot-packed length: 32 slots x 64
M = D * D                # 4096 flat matrix
B = 8192
NCORES = 8
RB = B // NCORES         # 1024 rows per core
P = 128                  # partitions
NT = RB // P             # 8 row-tiles per core
G = 2                    # tiles per compute group
NG = NT // G             # 4 groups
ETA = 0.05
RADIUS = 0.693

_IU = np.triu_indices(D, 1)

LAST_EXEC_NS = None
_NC_CACHE = {}

# ---------------------------------------------------------------------------
# Packing tables. Slot s (s=0..31, width 64) holds strip s (row s, cols
# s+1..63: 63-s values) followed by strip 62-s (s+1 values); slot 31 is
# strip 31 + 32 pad. Total 2048 (vs 2016 vec) but every slot is fixed-width,
# which makes both unvec directions two rectangular strided copies.
# ---------------------------------------------------------------------------
_off = np.zeros(D, np.int64)
for _i in range(1, D):
    _off[_i] = _off[_i - 1] + (D - _i)

IDX_PACK = np.zeros(S, np.int64)
_valid = np.zeros(S, bool)
for _s in range(32):
    _L1 = 63 - _s
    IDX_PACK[_s * 64:_s * 64 + _L1] = _off[_s] + np.arange(_L1)
    _valid[_s * 64:_s * 64 + _L1] = True
    if _s < 31:
        _t = 62 - _s
        IDX_PACK[_s * 64 + _L1:_s * 64 + 64] = _off[_t] + np.arange(_s + 1)
        _valid[_s * 64 + _L1:_s * 64 + 64] = True

INV = np.zeros(K, np.int64)
INV[IDX_PACK[_valid]] = np.nonzero(_valid)[0]

# upper-triangular mask over flat [64,64], replicated across 128 partitions
_mup = np.zeros(M, np.float32)
_mup[_IU[0] * D + _IU[1]] = 1.0
MASK_UP = np.ascontiguousarray(np.broadcast_to(np.tile(_mup, G), (P, G * M))).astype(bf16)

# select mask: position (s,e) valid for part1 iff e < 63-s
_msel = np.zeros(S, np.float32)
for _s in range(32):
    _msel[_s * 64:_s * 64 + (63 - _s)] = 1.0
MASK_SEL = np.ascontiguousarray(np.broadcast_to(np.tile(_msel, G), (P, G * S))).astype(bf16)


def _ap(base, ap_dims, offset):
    """Custom strided AP over an SBUF tensor's flat [P, n] view."""
    return dataclasses.replace(base, ap=[base.ap[0]] + ap_dims, offset=offset)


def _build_nc():
    nc = bass.Bass()
    vp = nc.dram_tensor("vp", [RB, S], mybir.dt.bfloat16, kind="ExternalInput")
    dp = nc.dram_tensor("dp", [RB, S], mybir.dt.bfloat16, kind="ExternalInput")
    mup = nc.dram_tensor("mup", [P, G * M], mybir.dt.bfloat16, kind="ExternalInput")
    msel = nc.dram_tensor("msel", [P, G * S], mybir.dt.bfloat16, kind="ExternalInput")
    wp = nc.dram_tensor("wp", [RB, S], mybir.dt.bfloat16, kind="ExternalOutput")

    dt = mybir.dt.bfloat16
    mult = mybir.AluOpType.mult
    add = mybir.AluOpType.add
    sub = mybir.AluOpType.subtract

    with (
        nc.sbuf_tensor([P, G * M], dt) as sMu,
        nc.sbuf_tensor([P, G * S], dt) as sMs,
        nc.sbuf_tensor([P, G * S], dt) as Vb,
        nc.sbuf_tensor([P, G * S], dt) as Db,
        nc.sbuf_tensor([P, G * S], dt) as Wv,
        nc.sbuf_tensor([P, G * M], dt) as UA,   # Up_A, later MAC tmp
        nc.sbuf_tensor([P, G * M], dt) as UD,   # Up_D, later P accumulator
        nc.sbuf_tensor([P, G * M], dt) as TA,   # A, later W
        nc.sbuf_tensor([P, G * M], dt) as TD,   # D
        nc.semaphore() as s_in,
        nc.semaphore() as s_mask,
        nc.semaphore() as s_v,
        nc.semaphore() as s_out,
        nc.Block() as block,
    ):
        # flat + structured views
        def tview(t, inner):  # [P, G*inner] -> [P, G, inner]
            return t[:, :].rearrange("p (g e) -> p g e", e=inner)

        Vb4 = Vb[:, :].rearrange("p (g s e) -> p g s e", s=32, e=64)
        Db4 = Db[:, :].rearrange("p (g s e) -> p g s e", s=32, e=64)
        Wv4 = Wv[:, :].rearrange("p (g s e) -> p g s e", s=32, e=64)

        A4 = TA[:, :].rearrange("p (g i j) -> p g i j", i=D, j=D)
        D4 = TD[:, :].rearrange("p (g i j) -> p g i j", i=D, j=D)
        P4 = UD[:, :].rearrange("p (g i j) -> p g i j", i=D, j=D)
        T4 = UA[:, :].rearrange("p (g i j) -> p g i j", i=D, j=D)

        # unvec target views on a flat base
        UAf = UA[:, :]
        UDf = UD[:, :]
        TAf = TA[:, :]

        @block.sync
        def _(sync):
            sync.dma_start(out=sMu[:, :], in_=mup[:, :]).then_inc(s_mask, 16)
            sync.dma_start(out=sMs[:, :], in_=msel[:, :]).then_inc(s_mask, 16)
            for g in range(NG):
                if g > 0:
                    sync.wait_ge(s_v, g)  # vector done with Vb/Db of group g-1
                rows = slice(g * G * P, (g + 1) * G * P)
                sync.dma_start(
                    out=tview(Vb, S),
                    in_=vp[rows, :].rearrange("(g p) e -> p g e", p=P),
                ).then_inc(s_in, 16)
                sync.dma_start(
                    out=tview(Db, S),
                    in_=dp[rows, :].rearrange("(g p) e -> p g e", p=P),
                ).then_inc(s_in, 16)

        @block.vector
        def _(vector):
            def dr():
                vector.drain()

            def unvec(upf, up_struct, src4):
                # memset; part2 (rows 31..62 full-width; strip-s dup into
                # lower, masked later); then part1 (rows 0..31 upper; also
                # fixes row 31 upper over part2's slot-31 pad garbage).
                vector.memset(upf, 0.0)
                dr()
                out_p2 = up_struct[:, :, 31:63, :]          # [p g 32 64] rows 31..62
                in_p2 = src4[:, :, 31::-1, :]               # slots 31..0
                vector.tensor_copy(out_p2, in_p2)
                dr()
                out_p1 = _ap(upf, [[M, G], [65, 32], [1, 64]], 1)
                in_p1 = src4
                vector.tensor_copy(out_p1, in_p1)
                dr()
                # mask to strictly-upper
                vector.tensor_tensor(upf, upf, sMu[:, :], mult)
                dr()

            UPA = UA[:, :].rearrange("p (g r e) -> p g r e", r=D, e=D)
            UPD = UD[:, :].rearrange("p (g r e) -> p g r e", r=D, e=D)

            vector.wait_ge(s_mask, 32)
            for g in range(NG):
                vector.wait_ge(s_in, 32 * (g + 1))
                if g > 0:
                    vector.wait_ge(s_out, 16 * g)  # prior store drained

                unvec(UA[:, :], UPA, Vb4)
                # A = Up - Up^T
                ua = UA[:, :].rearrange("p (g i j) -> p g i j", i=D, j=D)
                vector.tensor_tensor(A4, ua, ua.transpose([0, 1, 3, 2]), sub)
                dr()
                unvec(UD[:, :], UPD, Db4)
                ud = UD[:, :].rearrange("p (g i j) -> p g i j", i=D, j=D)
                vector.tensor_tensor(D4, ud, ud.transpose([0, 1, 3, 2]), sub)
                dr()

                # MAC: P = A @ D, accumulated over k. UD is dead -> P, UA -> tmp.
                a0 = A4[:, :, :, 0].unsqueeze(3).broadcast_to([P, G, D, D])
                d0 = D4[:, :, 0, :].unsqueeze(2).broadcast_to([P, G, D, D])
                vector.tensor_tensor(P4, a0, d0, mult)
                dr()
                with vector.Fori(1, D) as k:
                    ak = A4[:, :, :, k].unsqueeze(3).broadcast_to([P, G, D, D])
                    dk = D4[:, :, k, :].unsqueeze(2).broadcast_to([P, G, D, D])
                    vector.tensor_tensor(T4, ak, dk, mult)
                    dr()
                    vector.tensor_tensor(P4, P4, T4, add)
                    dr()

                # W = P - P^T -> TA (A dead); prescale by eta/2
                vector.tensor_tensor(A4, P4, P4.transpose([0, 1, 3, 2]), sub)
                dr()
                vector.tensor_scalar_mul(TA[:, :], TA[:, :], 0.5 * ETA)
                dr()
                # extract: t2 -> Wv, t1 -> Vb (dead), predicated merge
                in_t2 = _ap(TAf, [[M, G], [-64, 32], [1, 64]], 3968)
                vector.tensor_copy(Wv4, in_t2)
                in_t1 = _ap(TAf, [[M, G], [65, 32], [1, 64]], 1)
                vector.tensor_copy(Vb4, in_t1)
                dr()
                vector.copy_predicated(
                    Wv[:, :], sMs[:, :].bitcast(mybir.dt.uint16), Vb[:, :]
                )
                dr()
                # w = eta * dvec + Wv
                wv = tview(Wv, S)
                db = tview(Db, S)
                vector.scalar_tensor_tensor(
                    wv, db, ETA, wv, mult, add
                ).then_inc(s_v, 1)

        @block.scalar
        def _(scalar):
            for g in range(NG):
                scalar.wait_ge(s_v, g + 1)
                rows = slice(g * G * P, (g + 1) * G * P)
                scalar.dma_start(
                    out=wp[rows, :].rearrange("(g p) e -> p g e", p=P),
                    in_=tview(Wv, S),
                ).then_inc(s_out, 16)

    return nc


def _get_runner():
    """Build (once) a cached jitted SPMD executor with device-resident masks
    and output-zero buffer. Re-jitting per call (as run_bass_kernel_spmd
    does) costs seconds of XLA compile + buffer churn per invocation."""
    if "fn" in _NC_CACHE:
        return _NC_CACHE["fn"]
    import jax
    import jax.numpy as jnp
    from jax.sharding import Mesh, NamedSharding, PartitionSpec
    from jax.experimental.shard_map import shard_map
    from concourse import bass2jax

    bass2jax.install_neuronx_cc_hook()
    if "nc" not in _NC_CACHE:
        _NC_CACHE["nc"] = _build_nc()
    nc = _NC_CACHE["nc"]

    out_avals = (jax.core.ShapedArray((RB, S), jnp.bfloat16),)
    in_names = ("vp", "dp", "mup", "msel", "wp", "partition_id")

    def _body(vp_, dp_, mu_, ms_, wz_):
        outs = bass2jax._bass_exec_p.bind(
            vp_, dp_, mu_, ms_, wz_, bass2jax.partition_id_tensor(),
            out_avals=out_avals,
            in_names=in_names,
            out_names=("wp",),
            lowering_input_output_aliases=(),
            sim_require_finite=True,
            sim_require_nnan=True,
            nc=nc,
        )
        return outs[0]

    devices = jax.devices()[:NCORES]
    mesh = Mesh(np.asarray(devices), ("core",))
    spec = PartitionSpec("core")
    sh = NamedSharding(mesh, spec)
    sm = shard_map(
        _body, mesh=mesh, in_specs=(spec,) * 5, out_specs=spec, check_rep=False
    )
    fn = jax.jit(sm, in_shardings=(sh,) * 5, out_shardings=sh)
    mup_d = jax.device_put(np.tile(MASK_UP, (NCORES, 1)), sh)
    msel_d = jax.device_put(np.tile(MASK_SEL, (NCORES, 1)), sh)
    wz_d = jax.device_put(np.zeros((NCORES * RB, S), bf16), sh)
    _NC_CACHE["fn"] = (fn, mup_d, msel_d, wz_d)
    return _NC_CACHE["fn"]


def _exact_w_rows(vrows, drows):
    """Exact reference math (float64) for rows the cheap bounds can't settle.
    vrows: [n, K] f32 triu-vec of A_old; drows: [n, K] f32 triu-vec of dA.
    Returns w rows [n, K] f32 with w = vec(A_new) - vrows."""
    n = vrows.shape[0]
    A = np.zeros((n, D, D), np.float64)
    A[:, _IU[0], _IU[1]] = vrows.astype(np.float64)
    A -= A.transpose(0, 2, 1)
    dA = np.zeros((n, D, D), np.float64)
    dA[:, _IU[0], _IU[1]] = drows.astype(np.float64)
    dA -= dA.transpose(0, 2, 1)
    s_old = np.linalg.svd(A, compute_uv=False)[:, 0:1, None]
    s_del = np.linalg.svd(ETA * dA, compute_uv=False)[:, 0:1, None]
    avail = np.clip(RADIUS - s_old, 1e-8, None)
    scale = np.minimum(avail / (s_del + 1e-8), 1.0)
    dAs = dA * scale
    A_new = A + ETA * dAs + 0.5 * ETA * (np.matmul(A, dAs) - np.matmul(dAs, A))
    A_new = 0.5 * (A_new - A_new.transpose(0, 2, 1))
    s_new = np.linalg.svd(A_new, compute_uv=False)[:, 0:1, None]
    A_new = A_new * np.minimum(RADIUS / (s_new + 1e-8), 1.0)
    return (A_new[:, _IU[0], _IU[1]] - vrows.astype(np.float64)).astype(np.float32)


def kernel(**inputs):
    global LAST_EXEC_NS
    fib = np.ascontiguousarray(inputs["fiber_vectors"], dtype=np.float32)
    uid = np.asarray(inputs["user_ids"], dtype=np.int64)
    delta = np.ascontiguousarray(inputs["delta_A"], dtype=np.float32)

    # host: gather + skew-project
    V = fib[uid]                                                    # [B, K]
    dvec = 0.5 * (delta[:, _IU[0], _IU[1]] - delta[:, _IU[1], _IU[0]])

    # slot-pack to bf16 and launch the async H2D transfer + device work
    # first; the remaining host work below overlaps with it.
    vp = V[:, IDX_PACK].astype(bf16)
    dp = dvec[:, IDX_PACK].astype(bf16)

    trace = os.environ.get("KERNEL_TRACE", "0") == "1"
    if not trace:
        import jax
        fn, mup_d, msel_d, wz_d = _get_runner()
        sh = mup_d.sharding
        vpd = jax.device_put(vp, sh)
        dpd = jax.device_put(dp, sh)
        outg = fn(vpd, dpd, mup_d, msel_d, wz_d)  # async

    # rows where the Frobenius sufficient conditions can't prove that both
    # the BCH scale and the final clamp are exactly 1 -> exact host path
    fro_A = np.sqrt(2.0) * np.linalg.norm(V, axis=1)
    fro_dAe = ETA * np.sqrt(2.0) * np.linalg.norm(dvec, axis=1)
    hard = ((RADIUS - fro_A) < fro_dAe + 1e-6) | (
        fro_A + fro_dAe + fro_A * fro_dAe > RADIUS - 1e-6
    )
    hard_w = _exact_w_rows(V[hard], dvec[hard]) if hard.any() else None

    # full-output buffer while the device works
    out = fib.copy()

    if trace:
        # profiling path: per-call jit, but produces an NTFF trace
        if "nc" not in _NC_CACHE:
            _NC_CACHE["nc"] = _build_nc()
        in_maps = []
        for c in range(NCORES):
            rows = slice(c * RB, (c + 1) * RB)
            in_maps.append({
                "vp": vp[rows], "dp": dp[rows],
                "mup": MASK_UP, "msel": MASK_SEL,
            })
        res = bass_utils.run_bass_kernel_spmd(
            _NC_CACHE["nc"], in_maps, core_ids=list(range(NCORES)), trace=True,
        )
        LAST_EXEC_NS = res.exec_time_ns
        wpk = np.concatenate(
            [np.asarray(res.results[c]["wp"]) for c in range(NCORES)], axis=0
        ).astype(np.float32)
    else:
        wpk = np.asarray(outg).astype(np.float32)
        LAST_EXEC_NS = None

    wvec = wpk[:, INV]                                              # unpack

    if hard_w is not None:
        wvec[hard] = hard_w

    out[uid] = V + wvec
    return out
